# revision 14
# baseline (speedup 1.0000x reference)
"""ConvTransE forward on 8 Trainium2 NeuronCores (Bass/Tile) — v10 "relu fold".

Math: the reference returns out[b] = z[b] . ent[t[b]] with
z = relu(conv(x) + cb) @ proj_w.T + pb, x = [ent[h], rel[r][:,0-only]].
Because |ent| <= 0.0077 while conv_b ~ U(-0.58, 0.58), for most channels
relu is provably the identity (cb >= max possible |conv|) or provably
zero (cb <= -max).  Identity channels fold EXACTLY into one linear map
W_fold [513+ones, 512] built on host from the weights; zero channels
drop; only the few genuinely nonlinear channels keep the
conv->relu->proj path on device.  With the staged weights that is 23
linear / 8 zero / 1 nonlinear channel.

Sharding: data-parallel over batch.  Core m owns batch tiles 2m, 2m+1
(256 rows).  Per tile: indirect-gather ent[h] rows (bf16), PE-transpose
126-stride windows, z = xT.T @ W_fold (+ stub for x[503..512], rel,
ones/const rows), plus per-NL-channel banded conv (one matmul — the
band is segment-independent) + relu + projection, then
out[b] = z[b] . ent[t[b]] via fused multiply+row-sum.  No cross-core
reduction; host concatenates the per-core [TPC, 128] outputs.
rel[r][:,0] (8KB) is gathered on host.

v10 latency schedule (everything [128, few-cols] is poison — DMA
splits per-partition lines into 4..32-byte packets that trickle for
microseconds):
  - the output is PE-transposed to [TPC, 128] f32 before the store.
  - stubf/relv ride in w1 (early queue), pwT/stub5/bstub in w2.
  - dummy 512-col matmuls ramp the PE p-state (clock doubles after
    ~3.4us of activity) while the first gather is in flight.
"""

import numpy as np

NE, NRR, D, C, B = 100000, 500, 512, 32, 2048
NCORES = 8
NQ = B // 128              # 16 batch tiles of 128
TPC = NQ // NCORES         # 2 batch tiles per core
JB = 126                   # window stride; window s covers x[126s-1 .. 126s+126]
NWARM = 48                 # PE p-state warmup transposes

_CACHE = {}


def _build_nc(n_nl, cbvals):
    from contextlib import ExitStack

    import concourse.bass as bass
    import concourse.tile as tile
    from concourse import bacc, mybir
    from concourse.masks import make_identity

    f32 = mybir.dt.float32
    bf16 = mybir.dt.bfloat16
    i32 = mybir.dt.int32
    Alu = mybir.AluOpType

    nc = bacc.Bacc("TRN2", target_bir_lowering=False, debug=False,
                   num_devices=NCORES)

    # w1: [wfold 4*D | band n*126 | stubf D (11p) | relv TPC (128p)]
    # w2: [pwT n*4*D (126p) | stub5 n*D (8p) | bstub n*8 (10p)]
    W1 = 4 * D + n_nl * JB + D + TPC
    W2 = n_nl * 4 * D + n_nl * D + n_nl * 8
    ent = nc.dram_tensor("ent", [NE, D], bf16, kind="ExternalInput")
    idx = nc.dram_tensor("idx", [128, 2 * TPC], i32, kind="ExternalInput")
    w1 = nc.dram_tensor("w1", [128, W1], bf16, kind="ExternalInput")
    w2 = nc.dram_tensor("w2", [128, max(W2, 1)], bf16, kind="ExternalInput")
    out = nc.dram_tensor("out", [TPC, 128], f32, kind="ExternalOutput")

    with tile.TileContext(nc) as tc, ExitStack() as ctx:
        const = ctx.enter_context(tc.tile_pool(name="const", bufs=1))
        gpad_p = ctx.enter_context(tc.tile_pool(name="gpad", bufs=2))
        v_p = ctx.enter_context(tc.tile_pool(name="vt", bufs=2))
        gw_p = ctx.enter_context(tc.tile_pool(name="gw", bufs=2))
        ym_p = ctx.enter_context(tc.tile_pool(name="ym", bufs=2))
        ys_p = ctx.enter_context(tc.tile_pool(name="ys", bufs=2))
        sc_p = ctx.enter_context(tc.tile_pool(name="scr", bufs=2))
        tp_p = ctx.enter_context(tc.tile_pool(name="tp", bufs=2, space="PSUM"))
        ts_p = ctx.enter_context(tc.tile_pool(name="ts", bufs=1, space="PSUM"))
        y5_p = ctx.enter_context(tc.tile_pool(name="y5", bufs=1, space="PSUM"))
        yps_p = ctx.enter_context(tc.tile_pool(name="yps", bufs=1,
                                               space="PSUM"))
        z_p = ctx.enter_context(tc.tile_pool(name="zp", bufs=2, space="PSUM"))

        # index row first: the gathers depend only on this (single
        # partition -> single DMA packet, no per-line trickle)
        idx_sb = const.tile([128, 2 * TPC], i32)
        nc.sync.dma_start(idx_sb[:], idx[:], single_packet=True)
        # packed weights: two queues so the transfers overlap
        w1_sb = const.tile([128, W1], bf16)
        nc.sync.dma_start(w1_sb[:], w1[:])
        w2_sb = const.tile([128, max(W2, 1)], bf16)
        nc.scalar.dma_start(w2_sb[:], w2[:])

        wf_sb = w1_sb[:, 0:4 * D]
        band_sb = [w1_sb[:, 4 * D + i * JB:4 * D + (i + 1) * JB]
                   for i in range(n_nl)]
        o1 = 4 * D + n_nl * JB
        stf_sb = w1_sb[0:11, o1:o1 + D]
        relv_sb = w1_sb[:, o1 + D:o1 + D + TPC]
        pwT_sb = [w2_sb[0:JB, i * 4 * D:(i + 1) * 4 * D] for i in range(n_nl)]
        o = n_nl * 4 * D
        st5_sb = [w2_sb[0:8, o + i * D:o + (i + 1) * D] for i in range(n_nl)]
        o2 = o + n_nl * D
        bstub_sb = [w2_sb[0:10, o2 + i * 8:o2 + (i + 1) * 8]
                    for i in range(n_nl)]

        # gathers: h0 first and alone (its packets get the DMA
        # engines exclusively); the rest descgen only after h0 lands
        gpads, vts = [], []
        for btl in range(TPC):
            gpad = gpad_p.tile([128, 515], bf16, name="gpad")
            nc.vector.memset(gpad[:, 0:1], 0.0)
            nc.vector.memset(gpad[:, 514:515], 1.0)
            gpads.append(gpad)
        nc.gpsimd.indirect_dma_start(
            out=gpads[0][:, 1:513], out_offset=None, in_=ent[:],
            in_offset=bass.IndirectOffsetOnAxis(
                ap=idx_sb[:, 0:1], axis=0))
        gate = const.tile([1, 1], bf16)
        nc.gpsimd.tensor_copy(gate[:], gpads[0][0:1, 1:2])
        for btl in range(1, TPC):
            nc.gpsimd.indirect_dma_start(
                out=gpads[btl][:, 1:513], out_offset=None, in_=ent[:],
                in_offset=bass.IndirectOffsetOnAxis(
                    ap=idx_sb[:, btl:btl + 1], axis=0))
        for btl in range(TPC):
            vt = v_p.tile([128, D], f32, name="vt")
            nc.gpsimd.indirect_dma_start(
                out=vt[:], out_offset=None, in_=ent[:],
                in_offset=bass.IndirectOffsetOnAxis(
                    ap=idx_sb[:, TPC + btl:TPC + btl + 1], axis=0))
            vts.append(vt)

        ident = const.tile([128, 128], bf16)
        make_identity(nc, ident[:])
        cb_sb = []
        for i in range(n_nl):
            cbt = const.tile([128, 1], f32)
            nc.vector.memset(cbt[:], float(cbvals[i]))
            cb_sb.append(cbt)
        # preload the activation table so the first real relu is cheap
        actw = const.tile([1, 1], f32)
        nc.scalar.activation(actw[:], ident[0:1, 0:1],
                             mybir.ActivationFunctionType.Relu)
        out_sb = const.tile([128, TPC], f32)
        outT_sb = const.tile([TPC, 128], f32)

        # PE p-state warmup: dep-free transposes while the gather lands
        warmp = tp_p.tile([128, 512], bf16, tag="scr", bufs=1)
        for i in range(NWARM):
            nc.tensor.transpose(warmp[:, (i % 4) * 128:(i % 4 + 1) * 128],
                                ident[:], ident[:])

        ymss, ystss, zs = [], [], []
        for btl in range(TPC):
            gpad = gpads[btl]
            # rel value for x[512] rides in w1; ones via memset above
            nc.vector.tensor_copy(gpad[:, 513:514],
                                  relv_sb[:, btl:btl + 1])
            # transposes: 4 staggered 128-wide windows + 11-row stub
            tp = tp_p.tile([128, 512], bf16)
            for s in range(4):
                nc.tensor.transpose(tp[:, s * 128:(s + 1) * 128],
                                    gpad[:, JB * s:JB * s + 128], ident[:])
            tps = ts_p.tile([11, 128], bf16)
            nc.tensor.transpose(tps[:], gpad[:, 504:515], ident[:])
            gw = gw_p.tile([128, 640], bf16)
            nc.vector.tensor_copy(gw[:, 0:512], tp[:])
            nc.vector.tensor_copy(gw[0:11, 512:640], tps[:])

            # fold first (no relu dependency): z = xT.T @ W_fold
            z = z_p.tile([128, D], mybir.dt.float32, name="zt")
            for s in range(4):
                nc.tensor.matmul(z[:], gw[:, s * 128:(s + 1) * 128],
                                 wf_sb[:, s * D:(s + 1) * D],
                                 start=(s == 0), stop=False)
            nc.tensor.matmul(z[:], gw[0:11, 512:640], stf_sb, start=False,
                             stop=(n_nl == 0))
            zs.append(z)

            # nonlinear channels: banded conv (one matmul) + relu; the
            # relu latency hides under the next tile's transposes/fold
            yms, ysts = [], []
            for i in range(n_nl):
                y5p = y5_p.tile([JB, 512], mybir.dt.float32, name="y5p")
                nc.tensor.matmul(y5p[:], band_sb[i], gw[:, 0:512],
                                 start=True, stop=True)
                yps = yps_p.tile([8, 128], mybir.dt.float32, name="yps")
                nc.tensor.matmul(yps[:], bstub_sb[i], gw[0:10, 512:640],
                                 start=True, stop=True)
                ym = ym_p.tile([JB, 512], bf16, name="ym")
                nc.scalar.activation(ym[:], y5p[:],
                                     mybir.ActivationFunctionType.Relu,
                                     bias=cb_sb[i][0:JB, 0:1])
                yst = ys_p.tile([8, 128], bf16, name="yst")
                nc.scalar.activation(yst[:], yps[:],
                                     mybir.ActivationFunctionType.Relu,
                                     bias=cb_sb[i][0:8, 0:1])
                yms.append(ym)
                ysts.append(yst)
            ymss.append(yms)
            ystss.append(ysts)

        for btl in range(TPC):
            z = zs[btl]
            for i in range(n_nl):
                for s in range(4):
                    nc.tensor.matmul(z[:],
                                     ymss[btl][i][:, s * 128:(s + 1) * 128],
                                     pwT_sb[i][:, s * D:(s + 1) * D],
                                     start=False, stop=False)
                nc.tensor.matmul(z[:], ystss[btl][i][:], st5_sb[i],
                                 start=False, stop=(i == n_nl - 1))
            scr = sc_p.tile([128, D], bf16)
            nc.vector.scalar_tensor_tensor(
                out=scr[:], in0=z[:], scalar=1.0, in1=vts[btl][:],
                op0=Alu.mult, op1=Alu.mult,
                accum_out=out_sb[:, btl:btl + 1])

        # transpose the output so the store is [TPC, 128]: contiguous
        # 512B lines instead of 128 4-byte packets
        identf = const.tile([128, 128], mybir.dt.float32)
        nc.vector.tensor_copy(identf[:], ident[:])
        otp = tp_p.tile([TPC, 128], mybir.dt.float32, tag="scr", bufs=1)
        nc.tensor.transpose(otp[:], out_sb[:], identf[:])
        nc.vector.tensor_copy(outT_sb[:], otp[:])
        nc.sync.dma_start(out[:], outT_sb[:], single_packet=True)
    nc.finalize()
    return nc


def _host_prep(inputs):
    """Per-core input dicts + the exact relu fold, all from full inputs."""
    import ml_dtypes

    bf = ml_dtypes.bfloat16
    ent = np.asarray(inputs["ent"], dtype=np.float32)
    rel = np.asarray(inputs["rel"], dtype=np.float64)
    w = np.asarray(inputs["conv_w"], dtype=np.float64)     # [32, 1, 3]
    cb = np.asarray(inputs["conv_b"], dtype=np.float64)    # [32]
    pw = np.asarray(inputs["proj_w"], dtype=np.float64)    # [512, 16384]
    pb = np.asarray(inputs["proj_b"], dtype=np.float64)    # [512]
    h = np.asarray(inputs["h"]).astype(np.int32)
    r = np.asarray(inputs["r"]).astype(np.int32)
    t = np.asarray(inputs["t"]).astype(np.int32)

    ent_bf = np.ascontiguousarray(ent.astype(bf))

    # channel classification: relu provably identity / provably zero
    Me = float(np.abs(ent).max())
    Mr = float(np.abs(rel[:, 0]).max())
    aw = np.abs(w[:, 0, :])
    b_main = aw.sum(1) * Me
    b_last = (aw[:, 0] + aw[:, 1]) * Me + aw[:, 2] * Mr
    bound = np.maximum(b_main, b_last)
    lin = np.where(cb >= bound)[0]
    nl = np.where((cb < bound) & (cb > -bound))[0]
    n_nl = len(nl)

    # exact fold of linear channels: F [513, 512] over x, const [512]
    F = np.zeros((513, D))
    const = pb.copy()
    jg = np.arange(D)
    for c in lin:
        for k in range(3):
            i = jg + k - 1
            m = (i >= 0) & (i <= 512)
            F[i[m], :] += w[c, 0, k] * pw[:, c * D + jg[m]].T
        const += cb[c] * pw[:, c * D:(c + 1) * D].sum(1)

    # window scheme: window s partition p holds x[126s + p - 1]
    wfold = np.zeros((128, 4, D))
    for s in range(4):
        for p in range(JB):
            xi = JB * s + p - 1
            if 0 <= xi <= 502:
                wfold[p, s, :] = F[xi]
    stubf = np.zeros((11, D))
    stubf[0:10] = F[503:513]          # x[503..511] + rel row F[512]
    stubf[10] = const                 # ones row

    jl = np.arange(JB)
    jl8 = np.arange(8)
    bands, bstubs, pwTs, stub5s = [], [], [], []
    for c in nl:
        bd = np.zeros((128, JB))      # segment-independent band
        bs = np.zeros((10, 8))
        for k in range(3):
            bd[jl + k, jl] = w[c, 0, k]
            bs[jl8 + k, jl8] = w[c, 0, k]
        pT = np.zeros((JB, 4, D))
        for s in range(4):
            pT[:, s, :] = pw[:, c * D + JB * s: c * D + JB * (s + 1)].T
        bands.append(bd)
        bstubs.append(bs)
        pwTs.append(pT.reshape(JB, 4 * D))
        stub5s.append(pw[:, c * D + 504: c * D + 512].T)

    hI = h.reshape(NQ, 128)
    tI = t.reshape(NQ, 128)
    relv = rel[r, 0].astype(np.float32).reshape(NQ, 128).T

    # pack weights
    W1 = 4 * D + n_nl * JB + D + TPC
    W2 = n_nl * 4 * D + n_nl * D + n_nl * 8
    w1_common = np.zeros((128, W1))
    w1_common[:, 0:4 * D] = wfold.reshape(128, 4 * D)
    for i in range(n_nl):
        w1_common[:, 4 * D + i * JB:4 * D + (i + 1) * JB] = bands[i]
    o1 = 4 * D + n_nl * JB
    w1_common[0:11, o1:o1 + D] = stubf
    w2p = np.zeros((128, max(W2, 1)))
    for i in range(n_nl):
        w2p[0:JB, i * 4 * D:(i + 1) * 4 * D] = pwTs[i]
    o = n_nl * 4 * D
    for i in range(n_nl):
        w2p[0:8, o + i * D:o + (i + 1) * D] = stub5s[i]
    o2 = o + n_nl * D
    for i in range(n_nl):
        w2p[0:10, o2 + i * 8:o2 + (i + 1) * 8] = bstubs[i]
    w2p = np.ascontiguousarray(w2p.astype(bf))

    in_maps = []
    for m in range(NCORES):
        sl = slice(m * TPC, (m + 1) * TPC)
        idxm = np.concatenate([hI[sl].T, tI[sl].T], axis=1)
        w1p = w1_common.copy()
        w1p[:, o1 + D:o1 + D + TPC] = relv[:, sl]
        in_maps.append({
            "ent": ent_bf,
            "idx": np.ascontiguousarray(idxm),
            "w1": np.ascontiguousarray(w1p.astype(bf)),
            "w2": w2p,
        })
    return in_maps, n_nl, tuple(float(cb[c]) for c in nl)


def _run(inputs, trace=False, tmpdir=None):
    from concourse.bass_utils import run_bass_kernel_spmd

    in_maps, n_nl, cbvals = _host_prep(inputs)
    key = ("nc", n_nl, cbvals)
    if key not in _CACHE:
        _CACHE[key] = _build_nc(n_nl, cbvals)
    nc = _CACHE[key]
    res = run_bass_kernel_spmd(nc, in_maps, core_ids=list(range(NCORES)),
                               trace=trace, tmpdir=tmpdir)
    total = np.zeros((NQ, 128), np.float32)
    for m, mres in enumerate(res.results):
        total[m * TPC:(m + 1) * TPC, :] = mres["out"]
    return total.reshape(B), res


def kernel(**inputs):
    out, _ = _run(inputs, trace=False)
    return out


# revision 18
# speedup vs baseline: 1.0005x; 1.0005x over previous
"""ConvTransE forward on 8 Trainium2 NeuronCores (Bass/Tile) — v9 "relu fold".

Math: the reference returns out[b] = z[b] . ent[t[b]] with
z = relu(conv(x) + cb) @ proj_w.T + pb, x = [ent[h], rel[r][:,0-only]].
Because |ent| <= 0.0077 while conv_b ~ U(-0.58, 0.58), for most channels
relu is provably the identity (cb >= max possible |conv|) or provably
zero (cb <= -max).  Identity channels fold EXACTLY into one linear map
W_fold [513+ones, 512] built on host from the weights; zero channels
drop; only the few genuinely nonlinear channels keep the
conv->relu->proj path on device.  With the staged weights that is 23
linear / 8 zero / 1 nonlinear channel.

Sharding: data-parallel over batch.  Core m owns batch tiles 2m, 2m+1
(256 rows).  Per tile: indirect-gather ent[h] rows (bf16), PE-transpose
126-stride windows, z = xT.T @ W_fold (+ stub for x[503..512], rel,
ones/const rows), plus per-NL-channel banded conv (one matmul — the
band is segment-independent) + relu + projection, then
out[b] = z[b] . ent[t[b]] via fused multiply+row-sum.  No cross-core
reduction; host concatenates the per-core [128, 2] outputs.
rel[r][:,0] (8KB) is gathered on host.

v9 schedule: the first h-gather's index-load + descriptor-gen + 28
ns/row DMA keeps data away from the PE until ~14.5us while the Tile
preamble ends ~7.2us; that window is filled with dummy identity
transposes so the PE p-state ramps to full clock before real work.
Weights ride in two packed bf16 DMAs on two engine queues; conv bias
is baked as a memset; ACT table preloaded with a dummy activation.
"""

import numpy as np

NE, NRR, D, C, B = 100000, 500, 512, 32, 2048
NCORES = 8
NQ = B // 128              # 16 batch tiles of 128
TPC = NQ // NCORES         # 2 batch tiles per core
JB = 126                   # window stride; window s covers x[126s-1 .. 126s+126]
NWARM = 28                 # PE p-state warmup transposes

_CACHE = {}


def _build_nc(n_nl, cbvals):
    from contextlib import ExitStack

    import concourse.bass as bass
    import concourse.tile as tile
    from concourse import bacc, mybir
    from concourse.masks import make_identity

    f32 = mybir.dt.float32
    bf16 = mybir.dt.bfloat16
    i32 = mybir.dt.int32
    Alu = mybir.AluOpType

    nc = bacc.Bacc("TRN2", target_bir_lowering=False, debug=False,
                   num_devices=NCORES)

    # w1: [wfold 4*D | band n*126]  (128 partitions)
    # w2: [pwT n*4*D (126p) | stubf D (11p) | stub5 n*D (8p) | bstub n*8 (10p)
    #      | relv TPC (128p)]
    W1 = 4 * D + n_nl * JB
    W2 = n_nl * 4 * D + D + n_nl * D + n_nl * 8 + TPC
    ent = nc.dram_tensor("ent", [NE, D], bf16, kind="ExternalInput")
    idx = nc.dram_tensor("idx", [128, 2 * TPC], i32, kind="ExternalInput")
    w1 = nc.dram_tensor("w1", [128, W1], bf16, kind="ExternalInput")
    w2 = nc.dram_tensor("w2", [128, W2], bf16, kind="ExternalInput")
    out = nc.dram_tensor("out", [128, TPC], f32, kind="ExternalOutput")

    with tile.TileContext(nc) as tc, ExitStack() as ctx:
        const = ctx.enter_context(tc.tile_pool(name="const", bufs=1))
        gpad_p = ctx.enter_context(tc.tile_pool(name="gpad", bufs=2))
        v_p = ctx.enter_context(tc.tile_pool(name="vt", bufs=2))
        gw_p = ctx.enter_context(tc.tile_pool(name="gw", bufs=2))
        ym_p = ctx.enter_context(tc.tile_pool(name="ym", bufs=2))
        ys_p = ctx.enter_context(tc.tile_pool(name="ys", bufs=2))
        sc_p = ctx.enter_context(tc.tile_pool(name="scr", bufs=2))
        tp_p = ctx.enter_context(tc.tile_pool(name="tp", bufs=2, space="PSUM"))
        y5_p = ctx.enter_context(tc.tile_pool(name="y5", bufs=1, space="PSUM"))
        yps_p = ctx.enter_context(tc.tile_pool(name="yps", bufs=1,
                                               space="PSUM"))
        z_p = ctx.enter_context(tc.tile_pool(name="zp", bufs=2, space="PSUM"))

        # index table first: the gathers depend only on this
        idx_sb = const.tile([128, 2 * TPC], i32)
        nc.sync.dma_start(idx_sb[:], idx[:])
        # packed weights: two queues so the transfers overlap
        w1_sb = const.tile([128, W1], bf16)
        nc.sync.dma_start(w1_sb[:], w1[:])
        w2_sb = const.tile([128, W2], bf16)
        nc.scalar.dma_start(w2_sb[:], w2[:])

        wf_sb = w1_sb[:, 0:4 * D]
        band_sb = [w1_sb[:, 4 * D + i * JB:4 * D + (i + 1) * JB]
                   for i in range(n_nl)]
        pwT_sb = [w2_sb[0:JB, i * 4 * D:(i + 1) * 4 * D] for i in range(n_nl)]
        o = n_nl * 4 * D
        stf_sb = w2_sb[0:11, o:o + D]
        st5_sb = [w2_sb[0:8, o + D + i * D:o + D + (i + 1) * D]
                  for i in range(n_nl)]
        o2 = o + D + n_nl * D
        bstub_sb = [w2_sb[0:10, o2 + i * 8:o2 + (i + 1) * 8]
                    for i in range(n_nl)]
        relv_sb = w2_sb[:, o2 + n_nl * 8:o2 + n_nl * 8 + TPC]

        # issue all gathers up front (gpsimd queue)
        gpads, vts = [], []
        for btl in range(TPC):
            gpad = gpad_p.tile([128, 515], bf16, name="gpad")
            nc.vector.memset(gpad[:, 0:1], 0.0)
            nc.vector.memset(gpad[:, 514:515], 1.0)
            nc.gpsimd.indirect_dma_start(
                out=gpad[:, 1:513], out_offset=None, in_=ent[:],
                in_offset=bass.IndirectOffsetOnAxis(
                    ap=idx_sb[:, btl:btl + 1], axis=0))
            gpads.append(gpad)
        for btl in range(TPC):
            vt = v_p.tile([128, D], f32, name="vt")
            nc.gpsimd.indirect_dma_start(
                out=vt[:], out_offset=None, in_=ent[:],
                in_offset=bass.IndirectOffsetOnAxis(
                    ap=idx_sb[:, TPC + btl:TPC + btl + 1], axis=0))
            vts.append(vt)

        ident = const.tile([128, 128], bf16)
        make_identity(nc, ident[:])
        cb_sb = []
        for i in range(n_nl):
            cbt = const.tile([128, 1], f32)
            nc.vector.memset(cbt[:], float(cbvals[i]))
            cb_sb.append(cbt)
        # preload the activation table so the first real relu is cheap
        actw = const.tile([1, 1], f32)
        nc.scalar.activation(actw[:], ident[0:1, 0:1],
                             mybir.ActivationFunctionType.Relu)
        out_sb = const.tile([128, TPC], f32)

        # PE p-state warmup: dep-free transposes while the gather lands
        warm = tp_p.tile([128, 640], bf16)
        for i in range(NWARM):
            nc.tensor.transpose(warm[:, (i % 4) * 128:(i % 4 + 1) * 128],
                                ident[:], ident[:])

        for btl in range(TPC):
            gpad = gpads[btl]
            # rel value for x[512] rides in w2; ones via memset above
            nc.vector.tensor_copy(gpad[:, 513:514],
                                  relv_sb[:, btl:btl + 1])
            # transposes: 4 staggered 128-wide windows + 11-row stub
            tp = tp_p.tile([128, 640], bf16)
            for s in range(4):
                nc.tensor.transpose(tp[:, s * 128:(s + 1) * 128],
                                    gpad[:, JB * s:JB * s + 128], ident[:])
            nc.tensor.transpose(tp[0:11, 512:640], gpad[:, 504:515], ident[:])
            gw = gw_p.tile([128, 640], bf16)
            nc.vector.tensor_copy(gw[:], tp[:])

            # nonlinear channels: banded conv (one matmul) + relu
            yms, ysts = [], []
            for i in range(n_nl):
                y5p = y5_p.tile([JB, 512], mybir.dt.float32, name="y5p")
                nc.tensor.matmul(y5p[:], band_sb[i], gw[:, 0:512],
                                 start=True, stop=True)
                yps = yps_p.tile([8, 128], mybir.dt.float32, name="yps")
                nc.tensor.matmul(yps[:], bstub_sb[i], gw[0:10, 512:640],
                                 start=True, stop=True)
                ym = ym_p.tile([JB, 512], bf16, name="ym")
                nc.scalar.activation(ym[:], y5p[:],
                                     mybir.ActivationFunctionType.Relu,
                                     bias=cb_sb[i][0:JB, 0:1])
                yst = ys_p.tile([8, 128], bf16, name="yst")
                nc.scalar.activation(yst[:], yps[:],
                                     mybir.ActivationFunctionType.Relu,
                                     bias=cb_sb[i][0:8, 0:1])
                yms.append(ym)
                ysts.append(yst)

            # z accumulation: fold + fold-stub + per-NL proj + NL-stub
            z = z_p.tile([128, D], mybir.dt.float32, name="zt")
            for s in range(4):
                nc.tensor.matmul(z[:], gw[:, s * 128:(s + 1) * 128],
                                 wf_sb[:, s * D:(s + 1) * D],
                                 start=(s == 0), stop=False)
            nc.tensor.matmul(z[:], gw[0:11, 512:640], stf_sb, start=False,
                             stop=(n_nl == 0))
            for i in range(n_nl):
                for s in range(4):
                    nc.tensor.matmul(z[:], yms[i][:, s * 128:(s + 1) * 128],
                                     pwT_sb[i][:, s * D:(s + 1) * D],
                                     start=False, stop=False)
                nc.tensor.matmul(z[:], ysts[i][:], st5_sb[i], start=False,
                                 stop=(i == n_nl - 1))

            scr = sc_p.tile([128, D], bf16)
            nc.vector.scalar_tensor_tensor(
                out=scr[:], in0=z[:], scalar=1.0, in1=vts[btl][:],
                op0=Alu.mult, op1=Alu.mult,
                accum_out=out_sb[:, btl:btl + 1])
            nc.sync.dma_start(out[:, btl:btl + 1], out_sb[:, btl:btl + 1])
    nc.finalize()
    return nc


def _host_prep(inputs):
    """Per-core input dicts + the exact relu fold, all from full inputs."""
    import ml_dtypes

    bf = ml_dtypes.bfloat16
    ent = np.asarray(inputs["ent"], dtype=np.float32)
    rel = np.asarray(inputs["rel"], dtype=np.float64)
    w = np.asarray(inputs["conv_w"], dtype=np.float64)     # [32, 1, 3]
    cb = np.asarray(inputs["conv_b"], dtype=np.float64)    # [32]
    pw = np.asarray(inputs["proj_w"], dtype=np.float64)    # [512, 16384]
    pb = np.asarray(inputs["proj_b"], dtype=np.float64)    # [512]
    h = np.asarray(inputs["h"]).astype(np.int32)
    r = np.asarray(inputs["r"]).astype(np.int32)
    t = np.asarray(inputs["t"]).astype(np.int32)

    ent_bf = np.ascontiguousarray(ent.astype(bf))

    # channel classification: relu provably identity / provably zero
    Me = float(np.abs(ent).max())
    Mr = float(np.abs(rel[:, 0]).max())
    aw = np.abs(w[:, 0, :])
    b_main = aw.sum(1) * Me
    b_last = (aw[:, 0] + aw[:, 1]) * Me + aw[:, 2] * Mr
    bound = np.maximum(b_main, b_last)
    lin = np.where(cb >= bound)[0]
    nl = np.where((cb < bound) & (cb > -bound))[0]
    n_nl = len(nl)

    # exact fold of linear channels: F [513, 512] over x, const [512]
    F = np.zeros((513, D))
    const = pb.copy()
    jg = np.arange(D)
    for c in lin:
        for k in range(3):
            i = jg + k - 1
            m = (i >= 0) & (i <= 512)
            F[i[m], :] += w[c, 0, k] * pw[:, c * D + jg[m]].T
        const += cb[c] * pw[:, c * D:(c + 1) * D].sum(1)

    # window scheme: window s partition p holds x[126s + p - 1]
    wfold = np.zeros((128, 4, D))
    for s in range(4):
        for p in range(JB):
            xi = JB * s + p - 1
            if 0 <= xi <= 502:
                wfold[p, s, :] = F[xi]
    stubf = np.zeros((11, D))
    stubf[0:10] = F[503:513]          # x[503..511] + rel row F[512]
    stubf[10] = const                 # ones row

    jl = np.arange(JB)
    jl8 = np.arange(8)
    bands, bstubs, pwTs, stub5s = [], [], [], []
    for c in nl:
        bd = np.zeros((128, JB))      # segment-independent band
        bs = np.zeros((10, 8))
        for k in range(3):
            bd[jl + k, jl] = w[c, 0, k]
            bs[jl8 + k, jl8] = w[c, 0, k]
        pT = np.zeros((JB, 4, D))
        for s in range(4):
            pT[:, s, :] = pw[:, c * D + JB * s: c * D + JB * (s + 1)].T
        bands.append(bd)
        bstubs.append(bs)
        pwTs.append(pT.reshape(JB, 4 * D))
        stub5s.append(pw[:, c * D + 504: c * D + 512].T)

    hI = np.ascontiguousarray(h.reshape(NQ, 128).T)
    tI = np.ascontiguousarray(t.reshape(NQ, 128).T)
    relv = rel[r, 0].astype(np.float32).reshape(NQ, 128).T

    # pack weights: w1 [128, 4D + n*126], w2 [128, n*4D + D + n*D + n*8 + TPC]
    W1 = 4 * D + n_nl * JB
    W2 = n_nl * 4 * D + D + n_nl * D + n_nl * 8 + TPC
    w1p = np.zeros((128, W1))
    w1p[:, 0:4 * D] = wfold.reshape(128, 4 * D)
    for i in range(n_nl):
        w1p[:, 4 * D + i * JB:4 * D + (i + 1) * JB] = bands[i]
    w2_common = np.zeros((128, W2))
    for i in range(n_nl):
        w2_common[0:JB, i * 4 * D:(i + 1) * 4 * D] = pwTs[i]
    o = n_nl * 4 * D
    w2_common[0:11, o:o + D] = stubf
    for i in range(n_nl):
        w2_common[0:8, o + D + i * D:o + D + (i + 1) * D] = stub5s[i]
    o2 = o + D + n_nl * D
    for i in range(n_nl):
        w2_common[0:10, o2 + i * 8:o2 + (i + 1) * 8] = bstubs[i]
    w1p = w1p.astype(bf)

    in_maps = []
    for m in range(NCORES):
        sl = slice(m * TPC, (m + 1) * TPC)
        idxm = np.concatenate([hI[:, sl], tI[:, sl]], axis=1)
        w2p = w2_common.copy()
        w2p[:, o2 + n_nl * 8:o2 + n_nl * 8 + TPC] = relv[:, sl]
        in_maps.append({
            "ent": ent_bf,
            "idx": np.ascontiguousarray(idxm),
            "w1": w1p,
            "w2": np.ascontiguousarray(w2p.astype(bf)),
        })
    return in_maps, n_nl, tuple(float(cb[c]) for c in nl)


def _run(inputs, trace=False, tmpdir=None):
    from concourse.bass_utils import run_bass_kernel_spmd

    in_maps, n_nl, cbvals = _host_prep(inputs)
    key = ("nc", n_nl, cbvals)
    if key not in _CACHE:
        _CACHE[key] = _build_nc(n_nl, cbvals)
    nc = _CACHE[key]
    res = run_bass_kernel_spmd(nc, in_maps, core_ids=list(range(NCORES)),
                               trace=trace, tmpdir=tmpdir)
    total = np.zeros((128, NQ), np.float32)
    for m, mres in enumerate(res.results):
        total[:, m * TPC:(m + 1) * TPC] = mres["out"]
    return total.T.reshape(B), res


def kernel(**inputs):
    out, _ = _run(inputs, trace=False)
    return out
